# revision 14
# baseline (speedup 1.0000x reference)
"""GCN edge-classifier kernel for Trainium2, 8 NeuronCores.

Math reduction: with NCLASS=2, softmax(logits)[e] = [sigmoid(d), 1-sigmoid(d)]
where d = D0[col_e] + D1[row_e],
  D0[v] = dinv[v]*(t_u[v] + A_u[v]) + (b1@wu + bfc0-bfc1),
  D1[v] = dinv[v]*(t_w[v] + A_w[v]) + (b1@ww),
  A[v]  = dinv[v] * (x[v] @ (W1 @ [wu|ww])),     (2 scalars per node)
  t[v]  = sum_{edges e: col_e==v} A[row_e],
  dinv  = rsqrt(1 + indegree),
  wu = Wfc[:64,0]-Wfc[:64,1], ww = Wfc[64:,0]-Wfc[64:,1].

Sharding: edges sharded across 8 cores by target (col) range of 12500 nodes.
Per core, nodes are ranked by descending in-degree; window g = ranks
[128g, 128(g+1)) across the 128 partitions. Each node's incoming edges sit
contiguously in its partition's row at columns [colstart[g], colstart[g]+deg),
padded to the window-common width K[g] with slots pointing at a zeroed table
row. Aggregation is a plain per-window tensor_reduce along the free axis.
Cross-node fetches use per-column indirect DMA (128 rows / instruction, the
hardware's limit: one offset per partition), from bf16 node tables that are
allgathered between the passes.
"""
import numpy as np

N = 100000
E = 1600000
NFEAT = 256
NSH = 12500           # nodes per core
NW = 98               # 128-rank windows per core
NPAD = NW * 128       # 12544
NFULL = NPAD * 8      # 100352
ZROW_LOCAL = 12543    # pad rank on every core; A row (p=127)*98+(g=97)
ZROW = NPAD * 7 + ZROW_LOCAL
NCHK = 16             # D1-table chunks for pass-3 ap_gather
CHK = NFULL // NCHK   # 6272 rows per chunk (int16-indexable)

_compiled = None
_compiled_key = None
_meta = None          # (Ks, colstart, NCOLT) from the last _pack


def _build(Ks, L):
    import concourse.bass as bass
    import concourse.bacc as bacc
    import concourse.mybir as mybir
    from concourse.tile import TileContext, add_dep_helper
    from concourse.masks import make_identity

    AluOp = mybir.AluOpType
    Act = mybir.ActivationFunctionType
    f32 = mybir.dt.float32
    bf16 = mybir.dt.bfloat16
    i32 = mybir.dt.int32

    colstart = np.concatenate([[0], np.cumsum(Ks)]).astype(int)
    NCOLT = int(colstart[-1])
    NI = 16 * L              # ap_gather positions per chunk (per 16-part group)
    NFAT = NCHK * NI         # fat output columns

    from concourse import library_config
    nc = bacc.Bacc('TRN2', target_bir_lowering=False, debug=False, num_devices=8,
                   num_swdge_queues=4)

    # inputs
    xT = nc.dram_tensor('xT', [NFEAT, NPAD], bf16, kind='ExternalInput')
    rt = nc.dram_tensor('rt', [128, NCOLT], i32, kind='ExternalInput')   # translated row ids (pad=ZROW)
    W1 = nc.dram_tensor('W1', [NFEAT, 64], f32, kind='ExternalInput')
    Wfc = nc.dram_tensor('Wfc', [128, 2], f32, kind='ExternalInput')
    b1 = nc.dram_tensor('b1', [64, 1], f32, kind='ExternalInput')
    bfc = nc.dram_tensor('bfc', [1, 2], f32, kind='ExternalInput')
    i16 = mybir.dt.int16
    rtg = nc.dram_tensor('rtg', [128, NCHK * L], i16, kind='ExternalInput')  # D1-chunk-local rows
    ctg = nc.dram_tensor('ctg', [128, NCHK * L], i16, kind='ExternalInput')  # local D0 rows
    out4 = nc.dram_tensor('out4', [128, NFAT, 2], mybir.dt.bfloat16, kind='ExternalOutput')

    # internal DRAM node tables (bf16); row of node with rank q on core k is
    # k*NPAD + (q%128)*NW + q//128 (partition-major) so the table write DMA
    # is one contiguous run per partition.
    A_loc = nc.dram_tensor('A_loc', [NPAD, 2], bf16)
    A_full = nc.dram_tensor('A_full', [NFULL, 2], bf16, addr_space='Shared')
    D_loc = nc.dram_tensor('D_loc', [NPAD, 1], f32)    # D1 only; D0 is core-local
    D_full = nc.dram_tensor('D_full', [NFULL, 1], f32, addr_space='Shared')

    with TileContext(nc) as tc:
        with tc.tile_pool(name='cst', bufs=1) as cst, \
             tc.tile_pool(name='ps', bufs=1, space='PSUM') as ps, \
             tc.tile_pool(name='psw', bufs=4, space='PSUM') as psw, \
             tc.tile_pool(name='big', bufs=1) as big, \
             tc.tile_pool(name='wrk', bufs=2) as wrk:

            # ---- big loads first: rt gates the deg phase ----
            rt_sb = big.tile([128, NCOLT], i32, tag='rt')
            nc.sync.dma_start(out=rt_sb[:], in_=rt[:, :])
            xlo = big.tile([128, NPAD], bf16, tag='xlo')
            xhi = big.tile([128, NPAD], bf16, tag='xhi')
            HP = NPAD // 2
            nc.sync.dma_start(out=xlo[:, 0:HP], in_=xT[0:128, 0:HP])
            nc.scalar.dma_start(out=xhi[:, 0:HP], in_=xT[128:256, 0:HP])
            nc.scalar.dma_start(out=xlo[:, HP:NPAD], in_=xT[0:128, HP:NPAD])
            nc.sync.dma_start(out=xhi[:, HP:NPAD], in_=xT[128:256, HP:NPAD])

            ident = cst.tile([128, 128], f32)
            make_identity(nc, ident[:])

            # ---- constants: wuw [64,2] = [wu|ww] ----
            wfct = cst.tile([128, 2], f32)
            nc.sync.dma_start(out=wfct[:], in_=Wfc[:, :])
            diff = cst.tile([128, 1], f32)
            nc.vector.tensor_tensor(out=diff[:], in0=wfct[:, 0:1], in1=wfct[:, 1:2], op=AluOp.subtract)
            wuw = cst.tile([64, 2], f32)
            nc.vector.tensor_copy(out=wuw[0:64, 0:1], in_=diff[0:64, 0:1])
            nc.sync.dma_start(out=wuw[0:64, 1:2], in_=diff[64:128, 0:1])

            # W1T [64, 256] via PE transpose
            w1a = cst.tile([128, 64], f32)
            w1b = cst.tile([128, 64], f32)
            nc.sync.dma_start(out=w1a[:], in_=W1[0:128, :])
            nc.sync.dma_start(out=w1b[:], in_=W1[128:256, :])
            w1t = cst.tile([64, 256], f32)
            pt = ps.tile([64, 128], f32, tag='cstp')
            nc.tensor.transpose(out=pt[:], in_=w1a[:], identity=ident[:])
            nc.vector.tensor_copy(out=w1t[:, 0:128], in_=pt[:])
            pt2 = ps.tile([64, 128], f32, tag='cstp')
            nc.tensor.transpose(out=pt2[:], in_=w1b[:], identity=ident[:])
            nc.vector.tensor_copy(out=w1t[:, 128:256], in_=pt2[:])

            # q = W1 @ wuw  -> bf16 q_lo/q_hi [128, 2] for the bf16 matvec
            q_lo = cst.tile([128, 2], bf16)
            q_hi = cst.tile([128, 2], bf16)
            pq = ps.tile([128, 128], f32, tag='cstp')
            nc.tensor.matmul(out=pq[:, 0:2], lhsT=w1t[:, 0:128], rhs=wuw[:], start=True, stop=True)
            nc.vector.tensor_copy(out=q_lo[:], in_=pq[:, 0:2])
            pq2 = ps.tile([128, 128], f32, tag='cstp')
            nc.tensor.matmul(out=pq2[:, 0:2], lhsT=w1t[:, 128:256], rhs=wuw[:], start=True, stop=True)
            nc.vector.tensor_copy(out=q_hi[:], in_=pq2[:, 0:2])

            # cbc [128,2]: col 0 = b1@wu + (bfc0-bfc1), col 1 = b1@ww
            b1t = cst.tile([64, 1], f32)
            nc.sync.dma_start(out=b1t[:], in_=b1[:, :])
            pb = ps.tile([128, 128], f32, tag='cstp')
            nc.tensor.matmul(out=pb[0:1, 0:2], lhsT=b1t[:], rhs=wuw[:], start=True, stop=True)
            bfct = cst.tile([1, 2], f32)
            nc.sync.dma_start(out=bfct[:], in_=bfc[:, :])
            cuw1 = cst.tile([1, 2], f32)
            nc.vector.tensor_copy(out=cuw1[:], in_=pb[0:1, 0:2])
            dbt = cst.tile([1, 1], f32)
            nc.vector.tensor_tensor(out=dbt[:], in0=bfct[0:1, 0:1], in1=bfct[0:1, 1:2], op=AluOp.subtract)
            nc.vector.tensor_tensor(out=cuw1[0:1, 0:1], in0=cuw1[0:1, 0:1], in1=dbt[:], op=AluOp.add)
            ones1 = cst.tile([1, 128], f32)
            nc.vector.memset(ones1[:], 1.0)
            pcb = ps.tile([128, 128], f32, tag='cstp')
            nc.tensor.matmul(out=pcb[:, 0:2], lhsT=ones1[:], rhs=cuw1[:], start=True, stop=True)
            cbc = cst.tile([128, 2], f32)
            nc.vector.tensor_copy(out=cbc[:], in_=pcb[:, 0:2])

            # ---- deg from pad mask (int compare, f32 mask out) ----
            mask = wrk.tile([128, NCOLT], f32, tag='mask')
            nc.vector.tensor_scalar(out=mask[:], in0=rt_sb[:], scalar1=ZROW,
                                    scalar2=None, op0=AluOp.not_equal)
            deg = big.tile([128, NW], f32, tag='deg')
            nc.vector.memset(deg[:], 0.0)
            for g in range(NW):
                c0, c1 = int(colstart[g]), int(colstart[g + 1])
                if c1 > c0:
                    nc.vector.tensor_reduce(out=deg[:, g:g + 1], in_=mask[:, c0:c1],
                                            axis=mybir.AxisListType.X, op=AluOp.add)
            sq = wrk.tile([128, NW], f32, tag='sq')
            nc.scalar.activation(out=sq[:], in_=deg[:], func=Act.Sqrt, bias=1.0, scale=1.0)
            dinv = big.tile([128, NW], f32, tag='dinv')
            nc.vector.reciprocal(out=dinv[:], in_=sq[:])

            # ---- A = dinv * (x @ q), per 128-rank window; bf16 table copy ----
            A_sb = big.tile([128, NW, 2], f32, tag='A')
            A_bf = big.tile([128, NW, 2], bf16, tag='Abf')
            for g in range(NW):
                pxq = psw.tile([128, 2], f32, tag='acc')
                nc.tensor.matmul(out=pxq[:], lhsT=xlo[:, 128 * g:128 * (g + 1)], rhs=q_lo[:], start=True, stop=False)
                nc.tensor.matmul(out=pxq[:], lhsT=xhi[:, 128 * g:128 * (g + 1)], rhs=q_hi[:], start=False, stop=True)
                if g % 2 == 0:
                    nc.vector.tensor_tensor(out=A_sb[:, g, :], in0=pxq[:],
                                            in1=dinv[:, g:g + 1].to_broadcast([128, 2]), op=AluOp.mult)
                else:
                    nc.scalar.activation(out=A_sb[:, g, :], in_=pxq[:], func=Act.Copy,
                                         scale=dinv[:, g:g + 1])
            nc.vector.tensor_copy(out=A_bf[:], in_=A_sb[:])
            wA = nc.sync.dma_start(out=A_loc.rearrange('(p f) c -> p f c', p=128), in_=A_bf[:])
            cc1 = nc.gpsimd.collective_compute(
                'AllGather', AluOp.bypass, replica_groups=[list(range(8))],
                ins=[A_loc[:, :]], outs=[A_full[:, :]])
            add_dep_helper(cc1.ins, wA.ins, True, 'allgather after A write')

            # ---- pass 2: per-column gather of A[row], reduce per window ----
            ap_big = big.tile([128, NCOLT, 2], bf16, tag='ap')
            for c in range(NCOLT):
                gi = nc.gpsimd.indirect_dma_start(
                    out=ap_big[:, c, :], out_offset=None, in_=A_full[:, :],
                    in_offset=bass.IndirectOffsetOnAxis(ap=rt_sb[:, c:c + 1], axis=0))
                add_dep_helper(gi.ins, cc1.ins, True, 'gather after allgather')
            t_sb = big.tile([128, NW, 2], f32, tag='t')
            nc.vector.memset(t_sb[:], 0.0)
            for g in range(NW):
                c0, c1 = int(colstart[g]), int(colstart[g + 1])
                if c1 > c0:
                    nc.vector.tensor_reduce(
                        out=t_sb[:, g, :], in_=ap_big[:, c0:c1, :].rearrange('p k c -> p c k'),
                        axis=mybir.AxisListType.X, op=AluOp.add)

            # ---- D tables ----
            D_sb = big.tile([128, NW, 2], f32, tag='D')
            D1c = big.tile([128, NW, 1], f32, tag='D1c')
            D0c = big.tile([128, NW, 1], f32, tag='D0c')
            nc.vector.tensor_tensor(out=D_sb[:], in0=t_sb[:], in1=A_sb[:], op=AluOp.add)
            for ch in range(2):
                nc.vector.tensor_tensor(out=D_sb[:, :, ch], in0=D_sb[:, :, ch], in1=dinv[:], op=AluOp.mult)
                nc.vector.tensor_scalar(out=D_sb[:, :, ch], in0=D_sb[:, :, ch],
                                        scalar1=cbc[:, ch:ch + 1], scalar2=None, op0=AluOp.add)
            nc.vector.tensor_copy(out=D1c[:, :, 0], in_=D_sb[:, :, 1])
            nc.vector.tensor_copy(out=D0c[:, :, 0], in_=D_sb[:, :, 0])
            wD = nc.sync.dma_start(out=D_loc.rearrange('(p f) c -> p f c', p=128), in_=D1c[:])
            cc2 = nc.gpsimd.collective_compute(
                'AllGather', AluOp.bypass, replica_groups=[list(range(8))],
                ins=[D_loc[:, :]], outs=[D_full[:, :]])
            add_dep_helper(cc2.ins, wD.ins, True, 'allgather after D write')

            # ---- pass 3: bulk ap_gather of D1[row] + D0[col], sigmoid ----
            # idx lists (int16, wrapped per 16-partition group) come from the host
            rtg_sb = big.tile([128, NCHK * L], i16, tag='rtg')
            nc.sync.dma_start(out=rtg_sb[:], in_=rtg[:, :])
            ctg_sb = big.tile([128, NCHK * L], i16, tag='ctg')
            nc.sync.dma_start(out=ctg_sb[:], in_=ctg[:, :])
            lib = nc.gpsimd.load_library(library_config.ap_gather)
            add_dep_helper(lib.ins, cc2.ins, True, 'lib swap after last collective')
            for m in range(NCHK):
                tbl = big.tile([128, CHK], f32, tag=('xlo' if m % 2 == 0 else 'xhi'))
                wt = (nc.sync if m % 2 == 0 else nc.scalar).dma_start(
                    out=tbl[:],
                    in_=D_full[m * CHK:(m + 1) * CHK, :].rearrange('r c -> c r').to_broadcast([128, CHK]))
                add_dep_helper(wt.ins, cc2.ins, True, 'tbl after allgather2')
                g1 = wrk.tile([128, NI, 1], f32, tag='g1')
                gi1 = nc.gpsimd.ap_gather(
                    out_ap=g1[:], in_ap=tbl[:].rearrange('p (n c) -> p n c', c=1),
                    idxs_ap=rtg_sb[:, m * L:(m + 1) * L],
                    channels=128, num_elems=CHK, d=1, num_idxs=NI)
                add_dep_helper(gi1.ins, lib.ins, True, 'gather after lib swap')
                g0 = wrk.tile([128, NI, 1], f32, tag='g0')
                gi0 = nc.gpsimd.ap_gather(
                    out_ap=g0[:], in_ap=D0c[:],
                    idxs_ap=ctg_sb[:, m * L:(m + 1) * L],
                    channels=128, num_elems=NW, d=1, num_idxs=NI)
                add_dep_helper(gi0.ins, lib.ins, True, 'gather after lib swap')
                nc.vector.tensor_tensor(out=g1[:, :, 0], in0=g1[:, :, 0], in1=g0[:, :, 0], op=AluOp.add)
                oww = wrk.tile([128, NI, 2], bf16, tag='oww')
                nc.scalar.activation(out=oww[:, :, 0], in_=g1[:, :, 0], func=Act.Sigmoid, scale=1.0)
                nc.vector.tensor_scalar(out=oww[:, :, 1], in0=oww[:, :, 0], scalar1=-1.0,
                                        scalar2=1.0, op0=AluOp.mult, op1=AluOp.add)
                (nc.sync if m % 2 == 0 else nc.scalar).dma_start(
                    out=out4[:, m * NI:(m + 1) * NI, :], in_=oww[:])

    nc.compile()
    return nc


def _pack(x, edge_index, W1, b1, Wfc, bfc):
    global _meta
    r = np.asarray(edge_index[0], dtype=np.int64)
    c = np.asarray(edge_index[1], dtype=np.int64)
    deg_all = np.bincount(c, minlength=N)

    # per-core degree-descending rank; translated table row per node
    pos = np.empty(N, dtype=np.int64)
    rank_of = np.empty(N, dtype=np.int64)
    Ks_cores = np.zeros((8, NW), dtype=np.int64)
    orders = []
    for k in range(8):
        d = deg_all[k * NSH:(k + 1) * NSH]
        order = np.argsort(-d, kind='stable')
        orders.append(order)
        rank = np.empty(NSH, dtype=np.int64)
        rank[order] = np.arange(NSH)
        rank_of[k * NSH:(k + 1) * NSH] = rank
        pos[k * NSH:(k + 1) * NSH] = k * NPAD + (rank % 128) * NW + rank // 128
        sd = d[order]
        for g in range(NW):
            lo = g * 128
            if lo < NSH:
                Ks_cores[k, g] = sd[lo]
    Ks = [int(v) for v in Ks_cores.max(axis=0)]
    colstart = np.concatenate([[0], np.cumsum(Ks)]).astype(int)
    NCOLT = int(colstart[-1])
    _meta = (tuple(Ks), colstart, NCOLT)

    cores_data = []
    order_e = np.argsort(c, kind='stable')
    sc = c[order_e]
    sr = r[order_e]
    spos = order_e

    in_maps = []
    unpack = []
    for k in range(8):
        lo, hi = np.searchsorted(sc, [k * NSH, (k + 1) * NSH])
        ck = sc[lo:hi]                     # global col ids, sorted
        rk = sr[lo:hi]
        pk = spos[lo:hi]
        # j = index of the edge within its node's contiguous run
        run_start = np.searchsorted(ck, ck, side='left')
        j = np.arange(len(ck)) - run_start
        rank = rank_of[ck]
        g = rank // 128
        p = rank % 128
        col = colstart[g] + j
        rtr = np.full((128, NCOLT), ZROW, dtype=np.int32)
        rtr[p, col] = pos[rk].astype(np.int32)
        cores_data.append((p, col, g, pos[rk], pk))
        # x in rank order (column index == rank), bf16, pad tail zero
        xk = np.zeros((NFEAT, NPAD), dtype=np.float32)
        xk[:, :NSH] = np.asarray(x[k * NSH:(k + 1) * NSH], dtype=np.float32)[orders[k]].T
        import ml_dtypes
        xk = xk.astype(ml_dtypes.bfloat16)
        in_maps.append({
            'xT': xk, 'rt': rtr,
            'W1': np.asarray(W1, np.float32),
            'Wfc': np.asarray(Wfc, np.float32),
            'b1': np.asarray(b1, np.float32).reshape(64, 1),
            'bfc': np.asarray(bfc, np.float32).reshape(1, 2),
        })

    # ---- v4: chunk-sorted per-16-partition ap_gather lists ----
    # j = rank of each slot within its (partition, chunk) bucket
    allj = []
    L = 0
    for (p, col, g, rext, pk) in cores_data:
        m = rext // CHK
        key = p.astype(np.int64) * NCHK + m
        o2 = np.argsort(key, kind='stable')
        ks = key[o2]
        run = np.searchsorted(ks, ks, side='left')
        j = np.zeros(len(ks), dtype=np.int64)
        j[o2] = np.arange(len(ks)) - run
        cnt = np.bincount(key, minlength=128 * NCHK)
        L = max(L, int(cnt.max()))
        allj.append(j)
    _meta_v4 = (L,)

    for ci, (p, col, g, rext, pk) in enumerate(cores_data):
        j = allj[ci]
        m = rext // CHK
        loc = rext % CHK
        q = p // 16
        s = p % 16
        i = s * L + j                      # position within the chunk's group list
        NI = 16 * L
        rtg_t = np.zeros((128, NCHK * L), dtype=np.int16)
        ctg_t = np.zeros((128, NCHK * L), dtype=np.int16)
        posmap4 = np.full((128, NCHK * NI), -1, dtype=np.int64)
        rowidx = 16 * q + (i % 16)
        colidx = m * L + (i // 16)
        rtg_t[rowidx, colidx] = loc.astype(np.int16)
        ctg_t[rowidx, colidx] = g.astype(np.int16)
        posmap4[p, m * NI + i] = pk
        in_maps[ci]['rtg'] = rtg_t
        in_maps[ci]['ctg'] = ctg_t
        unpack.append(posmap4)
    globals()['_L'] = L
    return in_maps, unpack


def kernel(x, edge_index, W1, b1, Wfc, bfc):
    global _compiled, _compiled_key
    from concourse import bass_utils
    in_maps, unpack = _pack(x, edge_index, W1, b1, Wfc, bfc)
    Ks, colstart, NCOLT = _meta
    key = (Ks, _L)
    if _compiled is None or _compiled_key != key:
        _compiled = _build(list(Ks), _L)
        _compiled_key = key
    res = bass_utils.run_bass_kernel_spmd(_compiled, in_maps, core_ids=list(range(8)))
    out = np.zeros((E, 2), dtype=np.float32)
    for k in range(8):
        o = np.asarray(res.results[k]['out4']).astype(np.float32)   # [128, NFAT, 2]
        pm = unpack[k]
        m = pm >= 0
        out[pm[m], 0] = o[:, :, 0][m]
        out[pm[m], 1] = o[:, :, 1][m]
    return out


# revision 19
# speedup vs baseline: 1.0539x; 1.0539x over previous
"""GCN edge-classifier kernel for Trainium2, 8 NeuronCores.

Math reduction: with NCLASS=2, softmax(logits)[e] = [sigmoid(d), 1-sigmoid(d)]
where d = D0[col_e] + D1[row_e],
  D0[v] = dinv[v]*(t_u[v] + A_u[v]) + (b1@wu + bfc0-bfc1),
  D1[v] = dinv[v]*(t_w[v] + A_w[v]) + (b1@ww),
  A[v]  = dinv[v] * (x[v] @ (W1 @ [wu|ww])),     (2 scalars per node)
  t[v]  = sum_{edges e: col_e==v} A[row_e],
  dinv  = rsqrt(1 + indegree),
  wu = Wfc[:64,0]-Wfc[:64,1], ww = Wfc[64:,0]-Wfc[64:,1].

Sharding: edges sharded across 8 cores by target (col) range of 12500 nodes.
Per core, nodes are ranked by descending in-degree; window g = ranks
[128g, 128(g+1)) across the 128 partitions. Each node's incoming edges sit
contiguously in its partition's row at columns [colstart[g], colstart[g]+deg),
padded to the window-common width K[g] with slots pointing at a zeroed table
row. Aggregation is a plain per-window tensor_reduce along the free axis.
Cross-node fetches use per-column indirect DMA (128 rows / instruction, the
hardware's limit: one offset per partition), from bf16 node tables that are
allgathered between the passes.
"""
import numpy as np

N = 100000
E = 1600000
NFEAT = 256
NSH = 12500           # nodes per core
NW = 98               # 128-rank windows per core
NPAD = NW * 128       # 12544
NFULL = NPAD * 8      # 100352
ZROW_LOCAL = 12543    # pad rank on every core; A row (p=127)*98+(g=97)
ZROW = NPAD * 7 + ZROW_LOCAL
NCHK = 16             # D1-table chunks for pass-3 ap_gather
CHK = NFULL // NCHK   # 6272 rows per chunk (int16-indexable)

_compiled = None
_compiled_key = None
_meta = None          # (Ks, colstart, NCOLT) from the last _pack


def _build(Ks, L):
    import concourse.bass as bass
    import concourse.bacc as bacc
    import concourse.mybir as mybir
    from concourse.tile import TileContext, add_dep_helper
    from concourse.masks import make_identity

    AluOp = mybir.AluOpType
    Act = mybir.ActivationFunctionType
    f32 = mybir.dt.float32
    bf16 = mybir.dt.bfloat16
    i32 = mybir.dt.int32

    colstart = np.concatenate([[0], np.cumsum(Ks)]).astype(int)
    NCOLT = int(colstart[-1])
    NI = 16 * L              # ap_gather positions per chunk (per 16-part group)
    NFAT = NCHK * NI         # fat output columns

    from concourse import library_config
    nc = bacc.Bacc('TRN2', target_bir_lowering=False, debug=False, num_devices=8,
                   num_swdge_queues=4)

    # inputs
    xT = nc.dram_tensor('xT', [NFEAT, NPAD], bf16, kind='ExternalInput')
    rt = nc.dram_tensor('rt', [128, NCOLT], i32, kind='ExternalInput')   # translated row ids (pad=ZROW)
    W1 = nc.dram_tensor('W1', [NFEAT, 64], f32, kind='ExternalInput')
    Wfc = nc.dram_tensor('Wfc', [128, 2], f32, kind='ExternalInput')
    b1 = nc.dram_tensor('b1', [64, 1], f32, kind='ExternalInput')
    bfc = nc.dram_tensor('bfc', [1, 2], f32, kind='ExternalInput')
    i16 = mybir.dt.int16
    rtg = nc.dram_tensor('rtg', [128, NCHK * L], i16, kind='ExternalInput')  # D1-chunk-local rows
    ctg = nc.dram_tensor('ctg', [128, NCHK * L], i16, kind='ExternalInput')  # local D0 rows
    out4 = nc.dram_tensor('out4', [128, NFAT, 2], mybir.dt.bfloat16, kind='ExternalOutput')

    # internal DRAM node tables (bf16); row of node with rank q on core k is
    # k*NPAD + (q%128)*NW + q//128 (partition-major) so the table write DMA
    # is one contiguous run per partition.
    A_loc = nc.dram_tensor('A_loc', [NPAD, 2], bf16)
    A_full = nc.dram_tensor('A_full', [NFULL, 2], bf16, addr_space='Shared')
    D_loc = nc.dram_tensor('D_loc', [NPAD, 1], f32)    # D1 only; D0 is core-local
    D_full = nc.dram_tensor('D_full', [NFULL, 1], f32, addr_space='Shared')

    with TileContext(nc) as tc:
        with tc.tile_pool(name='cst', bufs=1) as cst, \
             tc.tile_pool(name='ps', bufs=1, space='PSUM') as ps, \
             tc.tile_pool(name='psw', bufs=4, space='PSUM') as psw, \
             tc.tile_pool(name='big', bufs=1) as big, \
             tc.tile_pool(name='wrk', bufs=2) as wrk:

            # ---- big loads first: rt gates the deg phase ----
            rt_sb = big.tile([128, NCOLT], i32, tag='rt')
            nc.sync.dma_start(out=rt_sb[:], in_=rt[:, :])
            xlo = big.tile([128, NPAD], bf16, tag='xlo')
            xhi = big.tile([128, NPAD], bf16, tag='xhi')
            HP = NPAD // 2
            nc.sync.dma_start(out=xlo[:, 0:HP], in_=xT[0:128, 0:HP])
            nc.scalar.dma_start(out=xhi[:, 0:HP], in_=xT[128:256, 0:HP])
            nc.scalar.dma_start(out=xlo[:, HP:NPAD], in_=xT[0:128, HP:NPAD])
            nc.sync.dma_start(out=xhi[:, HP:NPAD], in_=xT[128:256, HP:NPAD])

            ident = cst.tile([128, 128], f32)
            make_identity(nc, ident[:])

            # ---- constants: wuw [64,2] = [wu|ww] ----
            wfct = cst.tile([128, 2], f32)
            nc.sync.dma_start(out=wfct[:], in_=Wfc[:, :])
            diff = cst.tile([128, 1], f32)
            nc.vector.tensor_tensor(out=diff[:], in0=wfct[:, 0:1], in1=wfct[:, 1:2], op=AluOp.subtract)
            wuw = cst.tile([64, 2], f32)
            nc.vector.tensor_copy(out=wuw[0:64, 0:1], in_=diff[0:64, 0:1])
            nc.sync.dma_start(out=wuw[0:64, 1:2], in_=diff[64:128, 0:1])

            # W1T [64, 256] via PE transpose
            w1a = cst.tile([128, 64], f32)
            w1b = cst.tile([128, 64], f32)
            nc.sync.dma_start(out=w1a[:], in_=W1[0:128, :])
            nc.sync.dma_start(out=w1b[:], in_=W1[128:256, :])
            w1t = cst.tile([64, 256], f32)
            pt = ps.tile([64, 128], f32, tag='cstp')
            nc.tensor.transpose(out=pt[:], in_=w1a[:], identity=ident[:])
            nc.vector.tensor_copy(out=w1t[:, 0:128], in_=pt[:])
            pt2 = ps.tile([64, 128], f32, tag='cstp')
            nc.tensor.transpose(out=pt2[:], in_=w1b[:], identity=ident[:])
            nc.vector.tensor_copy(out=w1t[:, 128:256], in_=pt2[:])

            # q = W1 @ wuw  -> bf16 q_lo/q_hi [128, 2] for the bf16 matvec
            q_lo = cst.tile([128, 2], bf16)
            q_hi = cst.tile([128, 2], bf16)
            pq = ps.tile([128, 128], f32, tag='cstp')
            nc.tensor.matmul(out=pq[:, 0:2], lhsT=w1t[:, 0:128], rhs=wuw[:], start=True, stop=True)
            nc.vector.tensor_copy(out=q_lo[:], in_=pq[:, 0:2])
            pq2 = ps.tile([128, 128], f32, tag='cstp')
            nc.tensor.matmul(out=pq2[:, 0:2], lhsT=w1t[:, 128:256], rhs=wuw[:], start=True, stop=True)
            nc.vector.tensor_copy(out=q_hi[:], in_=pq2[:, 0:2])

            # cbc [128,2]: col 0 = b1@wu + (bfc0-bfc1), col 1 = b1@ww
            b1t = cst.tile([64, 1], f32)
            nc.sync.dma_start(out=b1t[:], in_=b1[:, :])
            pb = ps.tile([128, 128], f32, tag='cstp')
            nc.tensor.matmul(out=pb[0:1, 0:2], lhsT=b1t[:], rhs=wuw[:], start=True, stop=True)
            bfct = cst.tile([1, 2], f32)
            nc.sync.dma_start(out=bfct[:], in_=bfc[:, :])
            cuw1 = cst.tile([1, 2], f32)
            nc.vector.tensor_copy(out=cuw1[:], in_=pb[0:1, 0:2])
            dbt = cst.tile([1, 1], f32)
            nc.vector.tensor_tensor(out=dbt[:], in0=bfct[0:1, 0:1], in1=bfct[0:1, 1:2], op=AluOp.subtract)
            nc.vector.tensor_tensor(out=cuw1[0:1, 0:1], in0=cuw1[0:1, 0:1], in1=dbt[:], op=AluOp.add)
            ones1 = cst.tile([1, 128], f32)
            nc.vector.memset(ones1[:], 1.0)
            pcb = ps.tile([128, 128], f32, tag='cstp')
            nc.tensor.matmul(out=pcb[:, 0:2], lhsT=ones1[:], rhs=cuw1[:], start=True, stop=True)
            cbc = cst.tile([128, 2], f32)
            nc.vector.tensor_copy(out=cbc[:], in_=pcb[:, 0:2])

            # ---- deg from pad mask (int compare, f32 mask out) ----
            mask = wrk.tile([128, NCOLT], f32, tag='mask')
            nc.vector.tensor_scalar(out=mask[:], in0=rt_sb[:], scalar1=ZROW,
                                    scalar2=None, op0=AluOp.not_equal)
            deg = big.tile([128, NW], f32, tag='deg')
            nc.vector.memset(deg[:], 0.0)
            for g in range(NW):
                c0, c1 = int(colstart[g]), int(colstart[g + 1])
                if c1 > c0:
                    nc.vector.tensor_reduce(out=deg[:, g:g + 1], in_=mask[:, c0:c1],
                                            axis=mybir.AxisListType.X, op=AluOp.add)
            sq = wrk.tile([128, NW], f32, tag='sq')
            nc.scalar.activation(out=sq[:], in_=deg[:], func=Act.Sqrt, bias=1.0, scale=1.0)
            dinv = big.tile([128, NW], f32, tag='dinv')
            nc.vector.reciprocal(out=dinv[:], in_=sq[:])

            # ---- A = dinv * (x @ q), per 128-rank window; bf16 table copy ----
            A_sb = big.tile([128, NW, 2], f32, tag='A')
            A_bf = big.tile([128, NW, 2], bf16, tag='Abf')
            for g in range(NW):
                pxq = psw.tile([128, 2], f32, tag='acc')
                nc.tensor.matmul(out=pxq[:], lhsT=xlo[:, 128 * g:128 * (g + 1)], rhs=q_lo[:], start=True, stop=False)
                nc.tensor.matmul(out=pxq[:], lhsT=xhi[:, 128 * g:128 * (g + 1)], rhs=q_hi[:], start=False, stop=True)
                if g % 2 == 0:
                    nc.vector.tensor_tensor(out=A_sb[:, g, :], in0=pxq[:],
                                            in1=dinv[:, g:g + 1].to_broadcast([128, 2]), op=AluOp.mult)
                else:
                    nc.scalar.activation(out=A_sb[:, g, :], in_=pxq[:], func=Act.Copy,
                                         scale=dinv[:, g:g + 1])
            nc.vector.tensor_copy(out=A_bf[:], in_=A_sb[:])
            wA = nc.sync.dma_start(out=A_loc.rearrange('(p f) c -> p f c', p=128), in_=A_bf[:])
            cc1 = nc.gpsimd.collective_compute(
                'AllGather', AluOp.bypass, replica_groups=[list(range(8))],
                ins=[A_loc[:, :]], outs=[A_full[:, :]])
            add_dep_helper(cc1.ins, wA.ins, True, 'allgather after A write')

            # ---- pass 2: per-column gather of A[row], reduce per window ----
            ap_big = big.tile([128, NCOLT, 2], bf16, tag='ap')
            for c in range(NCOLT):
                gi = nc.gpsimd.indirect_dma_start(
                    out=ap_big[:, c, :], out_offset=None, in_=A_full[:, :],
                    in_offset=bass.IndirectOffsetOnAxis(ap=rt_sb[:, c:c + 1], axis=0))
                add_dep_helper(gi.ins, cc1.ins, True, 'gather after allgather')
            t_sb = big.tile([128, NW, 2], f32, tag='t')
            nc.vector.memset(t_sb[:], 0.0)
            for g in range(NW):
                c0, c1 = int(colstart[g]), int(colstart[g + 1])
                if c1 > c0:
                    nc.vector.tensor_reduce(
                        out=t_sb[:, g, :], in_=ap_big[:, c0:c1, :].rearrange('p k c -> p c k'),
                        axis=mybir.AxisListType.X, op=AluOp.add)

            # ---- D tables ----
            D_sb = big.tile([128, NW, 2], f32, tag='D')
            D1c = big.tile([128, NW, 1], f32, tag='D1c')
            D0c = big.tile([128, NW, 1], f32, tag='D0c')
            nc.vector.tensor_tensor(out=D_sb[:], in0=t_sb[:], in1=A_sb[:], op=AluOp.add)
            for ch in range(2):
                nc.vector.tensor_tensor(out=D_sb[:, :, ch], in0=D_sb[:, :, ch], in1=dinv[:], op=AluOp.mult)
                nc.vector.tensor_scalar(out=D_sb[:, :, ch], in0=D_sb[:, :, ch],
                                        scalar1=cbc[:, ch:ch + 1], scalar2=None, op0=AluOp.add)
            nc.vector.tensor_copy(out=D1c[:, :, 0], in_=D_sb[:, :, 1])
            nc.vector.tensor_copy(out=D0c[:, :, 0], in_=D_sb[:, :, 0])
            wD = nc.sync.dma_start(out=D_loc.rearrange('(p f) c -> p f c', p=128), in_=D1c[:])
            cc2 = nc.gpsimd.collective_compute(
                'AllGather', AluOp.bypass, replica_groups=[list(range(8))],
                ins=[D_loc[:, :]], outs=[D_full[:, :]])
            add_dep_helper(cc2.ins, wD.ins, True, 'allgather after D write')

            # ---- pass 3: bulk ap_gather of D1[row] + D0[col], sigmoid ----
            # idx lists (int16, wrapped per 16-partition group) come from the host
            rtg_sb = big.tile([128, NCHK * L], i16, tag='rtg')
            nc.sync.dma_start(out=rtg_sb[:], in_=rtg[:, :])
            ctg_sb = big.tile([128, NCHK * L], i16, tag='ctg')
            nc.sync.dma_start(out=ctg_sb[:], in_=ctg[:, :])
            lib = nc.gpsimd.load_library(library_config.ap_gather)
            add_dep_helper(lib.ins, cc2.ins, True, 'lib swap after last collective')
            # software-pipelined: prefetch chunk m's table before emitting the
            # trailing ops of chunk m-1 so in-order engine FIFOs never stall
            # a broadcast behind a sigmoid-dependent store.
            tbls = {}
            trail = []

            def emit_trail(mm, g1, oww):
                nc.scalar.activation(out=oww[:, :, 0], in_=g1[:, :, 0], func=Act.Sigmoid, scale=1.0)
                nc.vector.tensor_scalar(out=oww[:, :, 1], in0=oww[:, :, 0], scalar1=-1.0,
                                        scalar2=1.0, op0=AluOp.mult, op1=AluOp.add)
                (nc.sync if mm % 2 == 0 else nc.scalar).dma_start(
                    out=out4[:, mm * NI:(mm + 1) * NI, :], in_=oww[:])

            def emit_tbl(mm):
                tbl = big.tile([128, CHK], f32, tag=('xlo' if mm % 2 == 0 else 'xhi'))
                wt = (nc.sync if mm % 2 == 0 else nc.scalar).dma_start(
                    out=tbl[:],
                    in_=D_full[mm * CHK:(mm + 1) * CHK, :].rearrange('r c -> c r').to_broadcast([128, CHK]))
                add_dep_helper(wt.ins, cc2.ins, True, 'tbl after allgather2')
                tbls[mm] = tbl

            emit_tbl(0)
            emit_tbl(1)
            for m in range(NCHK):
                tbl = tbls.pop(m)
                g1 = wrk.tile([128, NI, 1], f32, tag='g1')
                gi1 = nc.gpsimd.ap_gather(
                    out_ap=g1[:], in_ap=tbl[:].rearrange('p (n c) -> p n c', c=1),
                    idxs_ap=rtg_sb[:, m * L:(m + 1) * L],
                    channels=128, num_elems=CHK, d=1, num_idxs=NI)
                add_dep_helper(gi1.ins, lib.ins, True, 'gather after lib swap')
                g0 = wrk.tile([128, NI, 1], f32, tag='g0')
                gi0 = nc.gpsimd.ap_gather(
                    out_ap=g0[:], in_ap=D0c[:],
                    idxs_ap=ctg_sb[:, m * L:(m + 1) * L],
                    channels=128, num_elems=NW, d=1, num_idxs=NI)
                add_dep_helper(gi0.ins, lib.ins, True, 'gather after lib swap')
                nc.vector.tensor_tensor(out=g1[:, :, 0], in0=g1[:, :, 0], in1=g0[:, :, 0], op=AluOp.add)
                oww = wrk.tile([128, NI, 2], bf16, tag='oww')
                if m + 2 < NCHK:
                    emit_tbl(m + 2)
                trail.append((m, g1, oww))
                if len(trail) >= 2:
                    emit_trail(*trail.pop(0))
            for tr in trail:
                emit_trail(*tr)

    nc.compile()
    return nc


def _pack(x, edge_index, W1, b1, Wfc, bfc):
    global _meta
    r = np.asarray(edge_index[0], dtype=np.int64)
    c = np.asarray(edge_index[1], dtype=np.int64)
    deg_all = np.bincount(c, minlength=N)

    # per-core degree-descending rank; translated table row per node
    pos = np.empty(N, dtype=np.int64)
    rank_of = np.empty(N, dtype=np.int64)
    Ks_cores = np.zeros((8, NW), dtype=np.int64)
    orders = []
    for k in range(8):
        d = deg_all[k * NSH:(k + 1) * NSH]
        order = np.argsort(-d, kind='stable')
        orders.append(order)
        rank = np.empty(NSH, dtype=np.int64)
        rank[order] = np.arange(NSH)
        rank_of[k * NSH:(k + 1) * NSH] = rank
        pos[k * NSH:(k + 1) * NSH] = k * NPAD + (rank % 128) * NW + rank // 128
        sd = d[order]
        for g in range(NW):
            lo = g * 128
            if lo < NSH:
                Ks_cores[k, g] = sd[lo]
    Ks = [int(v) for v in Ks_cores.max(axis=0)]
    colstart = np.concatenate([[0], np.cumsum(Ks)]).astype(int)
    NCOLT = int(colstart[-1])
    _meta = (tuple(Ks), colstart, NCOLT)

    cores_data = []
    order_e = np.argsort(c, kind='stable')
    sc = c[order_e]
    sr = r[order_e]
    spos = order_e

    in_maps = []
    unpack = []
    for k in range(8):
        lo, hi = np.searchsorted(sc, [k * NSH, (k + 1) * NSH])
        ck = sc[lo:hi]                     # global col ids, sorted
        rk = sr[lo:hi]
        pk = spos[lo:hi]
        # j = index of the edge within its node's contiguous run
        run_start = np.searchsorted(ck, ck, side='left')
        j = np.arange(len(ck)) - run_start
        rank = rank_of[ck]
        g = rank // 128
        p = rank % 128
        col = colstart[g] + j
        rtr = np.full((128, NCOLT), ZROW, dtype=np.int32)
        rtr[p, col] = pos[rk].astype(np.int32)
        cores_data.append((p, col, g, pos[rk], pk))
        # x in rank order (column index == rank), bf16, pad tail zero
        xk = np.zeros((NFEAT, NPAD), dtype=np.float32)
        xk[:, :NSH] = np.asarray(x[k * NSH:(k + 1) * NSH], dtype=np.float32)[orders[k]].T
        import ml_dtypes
        xk = xk.astype(ml_dtypes.bfloat16)
        in_maps.append({
            'xT': xk, 'rt': rtr,
            'W1': np.asarray(W1, np.float32),
            'Wfc': np.asarray(Wfc, np.float32),
            'b1': np.asarray(b1, np.float32).reshape(64, 1),
            'bfc': np.asarray(bfc, np.float32).reshape(1, 2),
        })

    # ---- v4: chunk-sorted per-16-partition ap_gather lists ----
    # j = rank of each slot within its (partition, chunk) bucket
    allj = []
    L = 0
    for (p, col, g, rext, pk) in cores_data:
        m = rext // CHK
        key = p.astype(np.int64) * NCHK + m
        o2 = np.argsort(key, kind='stable')
        ks = key[o2]
        run = np.searchsorted(ks, ks, side='left')
        j = np.zeros(len(ks), dtype=np.int64)
        j[o2] = np.arange(len(ks)) - run
        cnt = np.bincount(key, minlength=128 * NCHK)
        L = max(L, int(cnt.max()))
        allj.append(j)
    _meta_v4 = (L,)

    for ci, (p, col, g, rext, pk) in enumerate(cores_data):
        j = allj[ci]
        m = rext // CHK
        loc = rext % CHK
        q = p // 16
        s = p % 16
        i = s * L + j                      # position within the chunk's group list
        NI = 16 * L
        rtg_t = np.zeros((128, NCHK * L), dtype=np.int16)
        ctg_t = np.zeros((128, NCHK * L), dtype=np.int16)
        posmap4 = np.full((128, NCHK * NI), -1, dtype=np.int64)
        rowidx = 16 * q + (i % 16)
        colidx = m * L + (i // 16)
        rtg_t[rowidx, colidx] = loc.astype(np.int16)
        ctg_t[rowidx, colidx] = g.astype(np.int16)
        posmap4[p, m * NI + i] = pk
        in_maps[ci]['rtg'] = rtg_t
        in_maps[ci]['ctg'] = ctg_t
        unpack.append(posmap4)
    globals()['_L'] = L
    return in_maps, unpack


def kernel(x, edge_index, W1, b1, Wfc, bfc):
    global _compiled, _compiled_key
    from concourse import bass_utils
    in_maps, unpack = _pack(x, edge_index, W1, b1, Wfc, bfc)
    Ks, colstart, NCOLT = _meta
    key = (Ks, _L)
    if _compiled is None or _compiled_key != key:
        _compiled = _build(list(Ks), _L)
        _compiled_key = key
    res = bass_utils.run_bass_kernel_spmd(_compiled, in_maps, core_ids=list(range(8)))
    out = np.zeros((E, 2), dtype=np.float32)
    for k in range(8):
        o = np.asarray(res.results[k]['out4']).astype(np.float32)   # [128, NFAT, 2]
        pm = unpack[k]
        m = pm >= 0
        out[pm[m], 0] = o[:, :, 0][m]
        out[pm[m], 1] = o[:, :, 1][m]
    return out
